# revision 15
# baseline (speedup 1.0000x reference)
"""Single-head causal attention (V=K source bug) on 8 trn2 NeuronCores. v2.

Problem: x[4,2048,1024], W_Q/W_K/W_V[64,1024] (W_V unused by reference).
  Q = x @ W_Q.T ; K = x @ W_K.T ; V = K (reference bug)
  out = softmax(mask(Q K^T / sqrt(1024))) @ V      -> [4,2048,64]

Sharding: 2 cores per batch (core i: batch = i % 4, role r = i // 4).
Each batch's 8 query tiles of 256 rows split by parity (r=0 even, r=1 odd).
ONE SPMD graph; per-core differences folded into data:
 * x^T sent column-PERMUTED (own tiles first); causality over the permuted
   key order is uniform: own chunk 2s+h at slot s masks p+128h<=f
   (device affine_select, no mask DMA); oth chunks (2s,2s+1) at slot s
   are all-valid (r=1) or all-masked (r=0) -> scal 0/1 multiply.

v2 layout (vs v1): single sync HWDGE queue, dep-ordered col-half DMAs;
qT duplicated into rows 0-63 (qlow pass) and 64-127 (joint K|Q pass,
col-tiled) with no SBUF-SBUF dup DMA; kstk rows 0-63 = own K^T chunks,
rows 64-127 = oth (tile_position col packing); one [128,128] PE transpose
per chunk column yields V for own+oth together; outputs on the sync HW
queue. ScalarE runs only the 13-ACT exp chain.
"""

import sys

sys.path.insert(0, "/opt/trn_rl_repo")

import numpy as np
import ml_dtypes

BF16 = ml_dtypes.bfloat16

B, T, C, D = 4, 2048, 1024, 64
N_CORES = 8
QTILE = 256
N_SLOTS = 4
CHUNK = 128
SCALE = C ** -0.5
N_WARMUP = 32

TRACE = False
TRACE_CORES = None
LAST_RESULTS = None

# ---- unit table ----------------------------------------------------------
# cell = (side, chunk, slot): side 0 = own (kstk rows 0:64, rhs qT[0:64]),
# side 1 = oth (rows 64:128, rhs qT[64:128]).
# Units W1..W10: 4 slices of [128,256] in one PSUM tile [128, 4, 256].
# Each entry: (name, slices[4] of cell, exp_halves, diag, rmask)
#   exp_halves: list of (slice_lo, slice_hi, tier) - ACTIVATE granularity
#   diag: (slice_lo,) one affine_select over slices [lo, lo+2) or None
#   rmask: list of (slice_lo, n_slices) tensor_scalar muls
# tiers (DMA arrival): 0=s0a 1=s0b 2=s1 3=s2 4=s3


def _unit_defs():
    U = []
    U.append(("W1", [(0, 0, 0), (0, 1, 0), (0, 0, 1), (0, 1, 1)],
              [(0, 2, 0), (2, 4, 1)], 0, []))
    U.append(("W2", [(0, 2, 1), (0, 3, 1), (0, 2, 2), (0, 3, 2)],
              [(0, 2, 1), (2, 4, 2)], 0, []))
    U.append(("W3", [(0, 0, 2), (0, 1, 2), (0, 4, 2), (0, 5, 2)],
              [(0, 4, 2)], 2, []))
    U.append(("W4", [(0, 0, 3), (0, 1, 3), (0, 2, 3), (0, 3, 3)],
              [(0, 4, 2)], None, []))
    U.append(("W5", [(0, 4, 3), (0, 5, 3), (0, 6, 3), (0, 7, 3)],
              [(0, 4, 2)], 2, []))
    U.append(("W6", [(1, 0, 0), (1, 0, 1), (1, 1, 0), (1, 1, 1)],
              [(0, 4, 3)], None, [(0, 1), (2, 1)]))
    U.append(("W7", [(1, 0, 2), (1, 0, 3), (1, 1, 2), (1, 1, 3)],
              [(0, 4, 3)], None, []))
    U.append(("W8", [(1, 2, 1), (1, 2, 2), (1, 3, 1), (1, 3, 2)],
              [(0, 4, 3)], None, [(0, 1), (2, 1)]))
    U.append(("W9", [(1, 4, 2), (1, 4, 3), (1, 5, 2), (1, 5, 3)],
              [(0, 4, 4)], None, [(0, 1), (2, 1)]))
    U.append(("W10", [(1, 2, 3), (1, 3, 3), (1, 6, 3), (1, 7, 3)],
              [(0, 2, 3), (2, 4, 4)], None, [(2, 2)]))
    return U


# S^T matmuls per unit: list of (slice_lo, n_slices, side, chunk, slot_lo)
# wide (n_slices=2) only when the slices are the same chunk at adjacent
# slots AND arrive in one tier.
_ST_MMS = {
    "W1": [(0, 1, 0, 0, 0), (1, 1, 0, 1, 0), (2, 1, 0, 0, 1), (3, 1, 0, 1, 1)],
    "W2": [(0, 1, 0, 2, 1), (1, 1, 0, 3, 1), (2, 1, 0, 2, 2), (3, 1, 0, 3, 2)],
    "W3": [(0, 1, 0, 0, 2), (1, 1, 0, 1, 2), (2, 1, 0, 4, 2), (3, 1, 0, 5, 2)],
    "W4": [(0, 1, 0, 0, 3), (1, 1, 0, 1, 3), (2, 1, 0, 2, 3), (3, 1, 0, 3, 3)],
    "W5": [(0, 1, 0, 4, 3), (1, 1, 0, 5, 3), (2, 1, 0, 6, 3), (3, 1, 0, 7, 3)],
    "W6": [(0, 2, 1, 0, 0), (2, 2, 1, 1, 0)],
    "W7": [(0, 2, 1, 0, 2), (2, 2, 1, 1, 2)],
    "W8": [(0, 2, 1, 2, 1), (2, 2, 1, 3, 1)],
    "W9": [(0, 2, 1, 4, 2), (2, 2, 1, 5, 2)],
    "W10": [(0, 1, 1, 2, 3), (1, 1, 1, 3, 3), (2, 1, 1, 6, 3), (3, 1, 1, 7, 3)],
}


def _build_graph():
    import concourse.bass as bass
    import concourse.mybir as mybir
    import concourse.tile as tile
    from concourse import bacc
    from concourse.masks import make_identity
    from contextlib import ExitStack

    fp32 = mybir.dt.float32
    bf16 = mybir.dt.bfloat16
    EXP = mybir.ActivationFunctionType.Exp

    nc = bacc.Bacc(
        "TRN2",
        target_bir_lowering=False,
        debug=False,
        num_devices=N_CORES,
    )

    # host pre-arranges inputs partition-major so every DMA line is
    # 2-4KB contiguous per partition (256B/1KB lines cost ~25% DMA rate).
    xk = nc.dram_tensor(
        "xk", [8, CHUNK, C // CHUNK, 256], bf16, kind="ExternalInput"
    ).ap()
    wkq = nc.dram_tensor(
        "wkq", [CHUNK, C // CHUNK, 2 * D], bf16, kind="ExternalInput"
    ).ap()
    scald = nc.dram_tensor("scal", [CHUNK, 1], fp32, kind="ExternalInput").ap()
    out = nc.dram_tensor(
        "out", [D + 1, N_SLOTS * QTILE], fp32, kind="ExternalOutput"
    ).ap()

    CCH = C // CHUNK  # 8 contraction chunks
    assert xk.shape[2] == CCH
    units = _unit_defs()

    with tile.TileContext(nc) as tc, ExitStack() as ctx:
        consts = ctx.enter_context(tc.tile_pool(name="consts", bufs=1))
        xpool = ctx.enter_context(tc.tile_pool(name="xpool", bufs=1))
        kqpool = ctx.enter_context(tc.tile_pool(name="kqpool", bufs=1))
        ptpool = ctx.enter_context(tc.tile_pool(name="ptpool", bufs=10))
        opool = ctx.enter_context(tc.tile_pool(name="opool", bufs=4))
        psS = ctx.enter_context(tc.tile_pool(name="psS", bufs=2, space="PSUM"))
        psP = ctx.enter_context(tc.tile_pool(name="psP", bufs=2, space="PSUM"))
        psO = ctx.enter_context(tc.tile_pool(name="psO", bufs=1, space="PSUM"))

        # ---- constants / statics ----
        warm_src = consts.tile([128, 512], bf16)
        nc.vector.memset(warm_src, 0.0)
        warm_ps = psP.tile([128, 512], fp32, tag="proj", name="warm_ps")
        for w in range(N_WARMUP):
            nc.tensor.matmul(
                warm_ps[:, 0:128], lhsT=warm_src[:, 0:128],
                rhs=warm_src[:, 0:128],
                start=(w == 0), stop=(w == N_WARMUP - 1),
            )
        ident = consts.tile([128, 128], bf16)
        make_identity(nc, ident)
        warm = consts.tile([1, 1], fp32)
        nc.vector.memset(warm, 0.0)
        nc.scalar.activation(warm, warm, EXP)

        w_sb = consts.tile([128, CCH, 2 * D], bf16)
        scal_sb = consts.tile([128, 1], fp32)
        xs = [xpool.tile([128, CCH, 512], bf16, name=f"xslab{s}")
              for s in range(4)]

        # ---- input DMAs: one sync HWDGE queue, dependency order ----
        nc.sync.dma_start(out=scal_sb, in_=scald)
        nc.sync.dma_start(out=w_sb, in_=wkq)
        for b in range(8):
            s, h = b // 2, b % 2
            nc.sync.dma_start(
                out=xs[s][:, :, h * 256 : h * 256 + 256], in_=xk[b]
            )

        # ---- persistent SBUF tensors ----
        kstk = kqpool.tile([128, 1024], bf16)   # r0-63 own K^T, r64-127 oth
        qT = kqpool.tile([128, 1024], bf16)     # Q^T in both halves
        vones = kqpool.tile([128, CCH, 2, D + 2], bf16)
        nc.vector.memset(vones[:, :, :, D : D + 1], 1.0)

        # ---- engine-op emitters ----
        def joint_proj(s, c0, c1):
            """K|Q joint projection of cols [c0,c1) of slab s (slabs 0,1).
            rows 0-63 K^T -> kstk, rows 64-127 Q^T -> qT hi (both DVE)."""
            w = c1 - c0
            kq = psP.tile([128, 512], fp32, tag="proj", name=f"kq{s}{c0}")
            for c in range(CCH):
                nc.tensor.matmul(
                    kq[:, 0:w], lhsT=w_sb[:, c, :], rhs=xs[s][:, c, c0:c1],
                    start=(c == 0), stop=(c == CCH - 1),
                )
            dst = slice(s * 512 + c0, s * 512 + c1)
            ceng = nc.scalar if (s == 0 and c0 == 0) else nc.vector
            ceng.copy(kstk[0:64, dst], kq[0:64, 0:w]) if ceng is nc.scalar \
                else ceng.tensor_copy(kstk[0:64, dst], kq[0:64, 0:w])
            return kq

        def qthi(kq, s, c0, c1):
            """deferred Q^T hi copy (only oth-side S^T needs it)."""
            w = c1 - c0
            nc.vector.tensor_copy(
                qT[64:128, s * 512 + c0 : s * 512 + c1], kq[64:128, 0:w]
            )

        def qlow_proj(s, c0, c1):
            w = c1 - c0
            qp = psP.tile([128, 512], fp32, tag="proj", name=f"ql{s}{c0}")
            for c in range(CCH):
                nc.tensor.matmul(
                    qp[0:64, 0:w], lhsT=w_sb[:, c, D : 2 * D],
                    rhs=xs[s][:, c, c0:c1],
                    start=(c == 0), stop=(c == CCH - 1),
                )
            dst = slice(s * 512 + c0, s * 512 + c1)
            if s == 0 and c0 == 0:
                nc.scalar.copy(qT[0:64, dst], qp[0:64, 0:w])
            else:
                nc.vector.tensor_copy(qT[0:64, dst], qp[0:64, 0:w])

        def kproj_oth(s):
            """K^T of full slab s (2,3) -> kstk rows 64-127 (col-tiled via
            out base_partition 64)."""
            kp = psP.tile([128, 512], fp32, tag="proj", name=f"ko{s}")
            for c in range(CCH):
                nc.tensor.matmul(
                    kp[64:128, :], lhsT=w_sb[:, c, 0:D], rhs=xs[s][:, c, :],
                    start=(c == 0), stop=(c == CCH - 1),
                )
            d0 = (s - 2) * 512
            nc.vector.tensor_copy(kstk[64:128, d0 : d0 + 256], kp[64:128, 0:256])
            nc.vector.tensor_copy(kstk[64:128, d0 + 256 : d0 + 512], kp[64:128, 256:512])

        s_tiles = {}
        pt_tiles = {}

        def st_mms(uname, tiers):
            """emit this unit's S^T matmuls whose tier is in `tiers`."""
            name_u = uname
            if name_u not in s_tiles:
                s_tiles[name_u] = psS.tile(
                    [128, 4, 256], fp32, tag="s", name=f"sps_{name_u}"
                )
            sp = s_tiles[name_u]
            udef = next(u for u in units if u[0] == name_u)
            slices = udef[1]
            halves = udef[2]

            def slice_tier(sl):
                for lo, hi, t in halves:
                    if lo <= sl < hi:
                        return t
                raise AssertionError

            for (slo, nsl, side, ch, slot_lo) in _ST_MMS[name_u]:
                if slice_tier(slo) not in tiers:
                    continue
                rl = slice(64 * side, 64 * side + 64)
                nc.tensor.matmul(
                    sp[:, slo : slo + nsl, :].rearrange("p a q -> p (a q)"),
                    lhsT=kstk[rl, ch * CHUNK : (ch + 1) * CHUNK],
                    rhs=qT[rl, slot_lo * QTILE : (slot_lo + nsl) * QTILE],
                    start=True, stop=True,
                )

        def exp_half(uname, lo, hi):
            sp = s_tiles[uname]
            if uname not in pt_tiles:
                pt_tiles[uname] = ptpool.tile(
                    [128, 4, 256], bf16, tag="pt", name=f"pt_{uname}"
                )
            pt = pt_tiles[uname]
            nc.scalar.activation(
                pt[:, lo:hi, :].rearrange("p a q -> p (a q)"),
                sp[:, lo:hi, :].rearrange("p a q -> p (a q)"),
                EXP, scale=SCALE,
            )

        def diag_mask(uname, slo):
            pt = pt_tiles[uname]
            nc.gpsimd.affine_select(
                out=pt[:, slo : slo + 2, :],
                in_=pt[:, slo : slo + 2, :],
                compare_op=mybir.AluOpType.is_ge,
                fill=0.0,
                base=0,
                # keep where f - p - 128a >= 0  <=>  key p+128a <= query f
                pattern=[[-128, 2], [1, 256]],
                channel_multiplier=-1,
            )

        def rmask(uname, slo, nsl):
            pt = pt_tiles[uname]
            nc.vector.tensor_scalar_mul(
                pt[:, slo : slo + nsl, :].rearrange("p a q -> p (a q)"),
                pt[:, slo : slo + nsl, :].rearrange("p a q -> p (a q)"),
                scal_sb[:, 0:1],
            )

        def transp(c):
            """V (natural) for own chunk c and oth chunk c via one 128x128
            transpose of kstk column range c."""
            tp = psP.tile([128, 1024], bf16, tag="proj", name=f"tp{c}")
            nc.tensor.transpose(
                tp[:, 0:128],
                in_=kstk[:, c * CHUNK : (c + 1) * CHUNK],
                identity=ident,
            )
            nc.vector.tensor_copy(
                vones[:, c, :, 0:D],
                tp[:, 0:128].rearrange("p (a d) -> p a d", a=2),
            )

        o_ps_holder = {}
        pv_seen = {}
        o_init_done = {}
        pv_total = {}

        def pv_count():
            """precompute per-region PV totals from unit defs."""
            for u in units:
                for (side, ch, slot) in u[1]:
                    pv_total[slot] = pv_total.get(slot, 0) + 1

        pv_count()

        def pv(uname, which=None):
            """PV matmuls for unit's cells (which: filter by slice index)."""
            if "o" not in o_ps_holder:
                o_ps_holder["o"] = psO.tile(
                    [D + 1, N_SLOTS * QTILE], fp32, name="o_ps"
                )
                # one start=True zero-matmul per 512-col PSUM bank: the ONLY
                # start in each bank (start clears has_written bank-wide, so
                # interleaved per-region starts would drop accumulation).
                for bank in range(2):
                    nc.tensor.matmul(
                        o_ps_holder["o"][:, bank * 512 : (bank + 1) * 512],
                        lhsT=warm_src[:, 0 : D + 1],
                        rhs=warm_src,
                        start=True, stop=False, skip_group_check=True,
                    )
            o_ps = o_ps_holder["o"]
            udef = next(u for u in units if u[0] == uname)
            pt = pt_tiles[uname]
            for sl, (side, ch, slot) in enumerate(udef[1]):
                if which is not None and sl not in which:
                    continue
                seen = pv_seen.get(slot, 0)
                pv_seen[slot] = seen + 1
                nc.tensor.matmul(
                    o_ps[:, slot * QTILE : (slot + 1) * QTILE],
                    lhsT=vones[:, ch, side, 0 : D + 1],
                    rhs=pt[:, sl, :],
                    start=False,
                    stop=(seen + 1 == pv_total[slot]),
                    skip_group_check=True,
                )

        def close_region(j):
            o_ps = o_ps_holder["o"]
            o_sb = opool.tile([D + 1, QTILE], fp32, name=f"osb{j}")
            nc.vector.tensor_copy(
                o_sb, o_ps[:, j * QTILE : (j + 1) * QTILE]
            )
            nc.sync.dma_start(out=out[:, j * QTILE : (j + 1) * QTILE], in_=o_sb)

        # ---- emission schedule (queue order == dependency order) ----
        # tier 0: s0a
        kq0a = joint_proj(0, 0, 256)
        qlow_proj(0, 0, 256)
        st_mms("W1", {0})
        exp_half("W1", 0, 2)
        diag_mask("W1", 0)
        qthi(kq0a, 0, 0, 256)
        # tier 1: s0b -- W1h2 needs only qlow0b; W2h1 needs joint0b kstk
        qlow_proj(0, 256, 512)
        st_mms("W1", {1})
        exp_half("W1", 2, 4)
        kq0b = joint_proj(0, 256, 512)
        st_mms("W2", {1})
        exp_half("W2", 0, 2)
        diag_mask("W2", 0)
        qthi(kq0b, 0, 256, 512)
        # tier 2: s1 -- W2h2/W4 need only qlow1; W3/W5 need joint1 kstk
        qlow_proj(1, 0, 512)
        st_mms("W2", {2})
        st_mms("W4", {2})
        exp_half("W2", 2, 4)
        exp_half("W4", 0, 4)
        kq1 = joint_proj(1, 0, 512)
        st_mms("W3", {2})
        st_mms("W5", {2})
        exp_half("W3", 0, 4)
        diag_mask("W3", 2)
        exp_half("W5", 0, 4)
        diag_mask("W5", 2)
        qthi(kq1, 1, 0, 512)
        # tier 3: s2 -- scalar-feeding work first
        kproj_oth(2)
        st_mms("W6", {3})
        st_mms("W7", {3})
        st_mms("W8", {3})
        st_mms("W10", {3})
        exp_half("W6", 0, 4)
        exp_half("W7", 0, 4)
        exp_half("W8", 0, 4)
        exp_half("W10", 0, 2)
        # fill the pre-slab3 PE idle with transposes + early PVs
        transp(0)
        transp(1)
        transp(2)
        transp(3)
        rmask("W6", 0, 1)
        rmask("W6", 2, 1)
        pv("W1")
        pv("W2")
        # tier 4: scalar-feeding work ahead of the whole PV tail
        kproj_oth(3)
        st_mms("W9", {4})
        exp_half("W9", 0, 4)
        st_mms("W10", {4})
        exp_half("W10", 2, 4)
        # PV tail overlaps the last exps
        pv("W6")
        close_region(0)
        transp(4)
        transp(5)
        transp(6)
        transp(7)
        rmask("W8", 0, 1)
        rmask("W8", 2, 1)
        pv("W3")
        pv("W7")
        pv("W8")
        close_region(1)
        rmask("W9", 0, 1)
        rmask("W9", 2, 1)
        pv("W9")
        close_region(2)
        pv("W4")
        pv("W10", which={0, 1})
        rmask("W10", 2, 2)
        pv("W5")
        pv("W10", which={2, 3})
        close_region(3)

    nc.compile()
    return nc


_NC_CACHE = None


def _get_nc():
    global _NC_CACHE
    if _NC_CACHE is None:
        _NC_CACHE = _build_graph()
    return _NC_CACHE


def _perm_tiles(r):
    own = [2 * j + r for j in range(N_SLOTS)]
    oth = [2 * j + (1 - r) for j in range(N_SLOTS)]
    return own + oth


def _host_prep(x, W_Q, W_K):
    in_maps = []
    CCH = C // CHUNK
    wkq2 = np.concatenate([W_K.T, W_Q.T], axis=1).astype(BF16)  # [1024, 128]
    # [128, 8, 128]: wkq_pm[p, c, d] = wkq2[c*128 + p, d]
    wkq_pm = np.ascontiguousarray(wkq2.reshape(CCH, CHUNK, 2 * D).transpose(1, 0, 2))
    for i in range(N_CORES):
        b, r = i % B, i // B
        perm = _perm_tiles(r)
        xt = x[b].T.astype(BF16)  # [1024, 2048]
        cols = np.concatenate(
            [np.arange(QTILE * p, QTILE * p + QTILE) for p in perm]
        )
        xkt = xt[:, cols].reshape(CCH, CHUNK, T)  # [c, p, t]
        # 8 blocks [128, 8, 256]: block b2 = cols [b2*256, +256), partition-major
        xkb = np.ascontiguousarray(
            xkt.transpose(1, 0, 2)
            .reshape(CHUNK, CCH, 8, 256)
            .transpose(2, 0, 1, 3)
        )
        sc = np.full((CHUNK, 1), float(r), dtype=np.float32)
        in_maps.append({"xk": xkb, "wkq": wkq_pm, "scal": sc})
    return in_maps


def _ensure_ntff_hook():
    import types

    try:
        from antenv.axon_hooks import get_axon_ntff_profile_hook  # noqa: F401

        return
    except ImportError:
        pass
    import antenv

    mod = types.ModuleType("antenv.axon_hooks")
    mod._hook = None

    def set_axon_ntff_profile_hook(h):
        mod._hook = h

    def get_axon_ntff_profile_hook():
        return mod._hook

    mod.set_axon_ntff_profile_hook = set_axon_ntff_profile_hook
    mod.get_axon_ntff_profile_hook = get_axon_ntff_profile_hook
    sys.modules["antenv.axon_hooks"] = mod
    antenv.axon_hooks = mod
    try:
        from trn_agent_boot.trn_boot import _ntff_profile_via_ctypes

        hook = _ntff_profile_via_ctypes("/opt/axon/libaxon_pjrt.so")
        if hook is not None:
            set_axon_ntff_profile_hook(hook)
    except Exception as e:
        print(f"ntff hook install failed: {e}")


def kernel(x, W_Q, W_K, W_V=None, **_unused):
    global LAST_RESULTS
    if TRACE:
        _ensure_ntff_hook()
    x = np.asarray(x, dtype=np.float32)
    W_Q = np.asarray(W_Q, dtype=np.float32)
    W_K = np.asarray(W_K, dtype=np.float32)

    from concourse.bass_utils import run_bass_kernel_spmd

    nc = _get_nc()
    in_maps = _host_prep(x, W_Q, W_K)
    res = run_bass_kernel_spmd(
        nc,
        in_maps,
        core_ids=list(range(N_CORES)),
        trace=TRACE,
        trace_cores=TRACE_CORES,
    )
    LAST_RESULTS = res

    y = np.empty((B, T, D), dtype=np.float32)
    for i in range(N_CORES):
        b, r = i % B, i // B
        ot = res.results[i]["out"]  # [65, 1024]
        o = ot[0:D, :] / ot[D : D + 1, :]
        for j in range(N_SLOTS):
            t0 = QTILE * (2 * j + r)
            y[b, t0 : t0 + QTILE, :] = o[:, j * QTILE : (j + 1) * QTILE].T
    return y


# revision 16
# speedup vs baseline: 1.0774x; 1.0774x over previous
"""Single-head causal attention (V=K source bug) on 8 trn2 NeuronCores.

Problem: x[4,2048,1024], W_Q/W_K/W_V[64,1024] (W_V unused by reference).
  Q = x @ W_Q.T ; K = x @ W_K.T ; V = K (reference bug)
  out = softmax(mask(Q K^T / sqrt(1024))) @ V      -> [4,2048,64]

Sharding: 2 cores per batch (core i: batch = i % 4, role r = i // 4).
Each batch's 8 query tiles of 256 rows split by parity (r=0 even, r=1 odd).
ONE SPMD graph for all 8 cores. Per-core differences are folded into DATA:

 * x^T is sent column-PERMUTED, own query tiles first:
     positions 0..3 = own tiles (2j+r), positions 4..7 = other tiles.
   So the Q projection reads compile-time columns [0,1024); causality over
   the permuted key order is encoded in per-core 0/1 masks.
 * slot j (own tile 2j+r, query rows 256 of it) attends own chunks
   [0..2j+1] and other chunks [8..8+2j+1] (uniform r=1 shape; r=0 masks
   the over-provisioned tail) -> 4j+4 key chunks of 128.

Device pipeline (emission order ~= data-arrival order; one NEFF, no
collectives -- their latency floor exceeds the whole kernel):
 * HAM warmup + filler matmuls keep the PE at 2.4 GHz across DMA waits.
 * Projections as col-paired M=64+64 matmuls (Q pair; K stacks
   A = permuted cols 0-511 | 1024-1535, B = 512-1023 | 1536-2047),
   PSUM->SBUF casts on the idle ScalarE early / VectorE mid-chain.
 * Per 4-chunk group: S^T = K^T-chunk(stationary) x Q^T(moving), mixed
   groups row-packed (own chunk on array rows 0-63 concurrent with other
   chunk on rows 64-127); exp on ScalarE ([128,1024] PSUM->SBUF bf16,
   1/sqrt(C) folded into the activation scale; no max-subtraction --
   |scores| <= ~1 by construction). The serial ~11us ACT exp chain is
   the critical path; group order follows slab arrival so it never
   starves. Causal masks: elementwise MUL on own-diagonal chunks,
   scalar 0/1 MUL on the padded other-side chunks.
 * V natural (V=K) via PE transposes of K^T; PV matmuls use
   lhsT=[V|ones] so PSUM row 64 accumulates the softmax denominator.
 * Host divides by row 64 and transposes the [65,1024] outputs back.
"""

import os
import sys

sys.path.insert(0, "/opt/trn_rl_repo")

import numpy as np
import ml_dtypes

BF16 = ml_dtypes.bfloat16

B, T, C, D = 4, 2048, 1024, 64
N_CORES = 8
QTILE = 256          # query rows per slot
N_SLOTS = 4
CHUNK = 128          # key chunk
GROUP = 4            # chunks per exp group ([128, 4*256] psum tile)
SCALE = C ** -0.5
N_WARMUP = 80        # HAM warmup matmuls (cover the DMA wait before Q proj)

TRACE = False
TRACE_CORES = None
LAST_RESULTS = None


def _slot_groups_def(j):
    """Groups of 4 chunks for slot j with mask kind per group.
    Kinds: 'mixed' (slices 0-1 own diag MUL, 2-3 oth TS),
           'own_diag' (slices 2-3 MUL), 'oth_tail' (slices 2-3 TS),
           'plain'. Own chunks are 0..2j+1, other chunks 8..8+2j+1."""
    if j == 0:
        return [([0, 1, 8, 9], "mixed")]
    if j == 1:
        return [([0, 1, 2, 3], "own_diag"), ([8, 9, 10, 11], "oth_tail")]
    if j == 2:
        return [
            ([0, 1, 2, 3], "plain"),
            ([8, 9, 10, 11], "plain"),
            ([4, 5, 12, 13], "mixed"),
        ]
    return [
        ([0, 1, 2, 3], "plain"),
        ([8, 9, 10, 11], "plain"),
        ([4, 5, 6, 7], "own_diag"),
        ([12, 13, 14, 15], "oth_tail"),
    ]


def _chunk_stack(c):
    """abs permuted chunk c -> (stack_idx, half, within). Stack A covers
    permuted cols 0-511 (top) and 1024-1535 (bottom); B covers 512-1023
    (top) and 1536-2047 (bottom)."""
    pos = c // 2            # 256-col tile position 0..7
    if pos < 4:             # own side -> top halves
        return (pos // 2, 0, c % 4)
    else:                   # other side -> bottom halves
        return ((pos - 4) // 2, 1, c % 4)


def _build_graph():
    import concourse.bass as bass
    import concourse.mybir as mybir
    import concourse.tile as tile
    from concourse import bacc
    from concourse.masks import make_identity
    from contextlib import ExitStack

    fp32 = mybir.dt.float32
    bf16 = mybir.dt.bfloat16

    nc = bacc.Bacc(
        "TRN2",
        target_bir_lowering=False,
        debug=False,
        num_devices=N_CORES,
    )

    xkt = nc.dram_tensor("xkt", [C, T], bf16, kind="ExternalInput").ap()
    wkq = nc.dram_tensor("wkq", [C, 2 * D], bf16, kind="ExternalInput").ap()
    maskd = nc.dram_tensor(
        "mask", [CHUNK, 2 * N_SLOTS, QTILE], bf16, kind="ExternalInput"
    ).ap()
    scald = nc.dram_tensor(
        "scal", [CHUNK, N_SLOTS], fp32, kind="ExternalInput"
    ).ap()
    out = nc.dram_tensor(
        "out", [D + 1, N_SLOTS * QTILE], fp32, kind="ExternalOutput"
    ).ap()

    NQ = N_SLOTS * QTILE           # 1024 own query cols
    NCH = T // CHUNK               # 16 key chunks
    CCH = C // CHUNK               # 8 contraction chunks

    with tile.TileContext(nc) as tc, ExitStack() as ctx:
        consts = ctx.enter_context(tc.tile_pool(name="consts", bufs=1))
        xpool = ctx.enter_context(tc.tile_pool(name="xpool", bufs=1))
        kqpool = ctx.enter_context(tc.tile_pool(name="kqpool", bufs=1))
        ptpool = ctx.enter_context(tc.tile_pool(name="ptpool", bufs=10))
        opool = ctx.enter_context(tc.tile_pool(name="opool", bufs=2))
        psP = ctx.enter_context(tc.tile_pool(name="psP", bufs=2, space="PSUM"))
        psS = ctx.enter_context(tc.tile_pool(name="psS", bufs=2, space="PSUM"))
        psO = ctx.enter_context(tc.tile_pool(name="psO", bufs=2, space="PSUM"))

        # ---- constants ----
        # warmup matmuls on a memset tile: near-zero deps, start immediately
        warm_src = consts.tile([128, 128], bf16)
        nc.vector.memset(warm_src, 0.0)
        warm_ps = psP.tile([128, 128], fp32, tag="proj")
        for w in range(N_WARMUP):
            nc.tensor.matmul(
                warm_ps, lhsT=warm_src, rhs=warm_src,
                start=(w == 0), stop=(w == N_WARMUP - 1),
            )
        ident = consts.tile([128, 128], bf16)
        make_identity(nc, ident)
        warm = consts.tile([1, 1], fp32)
        nc.vector.memset(warm, 0.0)
        nc.scalar.activation(warm, warm, mybir.ActivationFunctionType.Exp)

        # ---- DMAs (slab order drives the pipeline) ----
        w_sb = consts.tile([128, CCH, 2 * D], bf16)
        nc.sync.dma_start(out=w_sb, in_=wkq.rearrange("(c p) d -> p c d", p=128))
        # xkt slabs: 4 x [128, CCH, 512] column slabs of the permuted x^T
        xs = []
        xkt_r = xkt.rearrange("(c p) t -> p c t", p=128)
        # interleave slab halves so Q (s0+s1) completes earliest, then A
        # (s0+s2), then B (s1+s3); single sync HWDGE queue (HBM-bound anyway)
        for s in range(4):
            xsl = xpool.tile([128, CCH, 512], bf16, name=f"xslab{s}")
            xs.append(xsl)

        def slab_dma(s, c0, c1, eng=None):
            (eng or nc.sync).dma_start(
                out=xs[s][:, c0:c1, :],
                in_=xkt_r[:, c0:c1, s * 512 : (s + 1) * 512],
            )

        for s in (0, 1, 2):
            slab_dma(s, 0, 4)
            slab_dma(s, 4, 8)
        mask_sb = consts.tile([128, 2 * N_SLOTS, QTILE], bf16)
        scal_sb = consts.tile([128, N_SLOTS], fp32)
        nc.sync.dma_start(out=scal_sb, in_=scald)
        nc.sync.dma_start(out=mask_sb, in_=maskd)
        # slab 3 in column halves: chunks 12-13 land before 14-15
        for q in range(2):
            nc.sync.dma_start(
                out=xs[3][:, :, q * 256 : (q + 1) * 256],
                in_=xkt_r[:, :, 3 * 512 + q * 256 : 3 * 512 + (q + 1) * 256],
            )

        # ---- Q projection (col-paired: slabs 0,1 -> psum halves) ----
        qT = kqpool.tile([128, NQ], bf16)   # Q^T duplicated in both halves

        def filler(n, tag):
            f_ps = psP.tile([128, 128], fp32, tag="proj", name=f"warmf_{tag}")
            for w in range(n):
                nc.tensor.matmul(
                    f_ps, lhsT=warm_src, rhs=warm_src,
                    start=(w == 0), stop=(w == n - 1),
                )

        def qproj():
            q_ps = psP.tile([128, 512], fp32, tag="proj", name="qps")
            for c in range(CCH):
                nc.tensor.matmul(
                    q_ps[0:64, :], lhsT=w_sb[:, c, D : 2 * D], rhs=xs[0][:, c, :],
                    start=(c == 0), stop=(c == CCH - 1),
                )
                nc.tensor.matmul(
                    q_ps[64:128, :], lhsT=w_sb[:, c, D : 2 * D], rhs=xs[1][:, c, :],
                    start=(c == 0), stop=(c == CCH - 1),
                )
            nc.scalar.copy(qT[0:64, 0:512], q_ps[0:64, :])
            nc.scalar.copy(qT[0:64, 512:1024], q_ps[64:128, :])
            # duplicate into partitions 64-127 (cross-partition -> DMA).
            # gpsimd queue: the sync HWDGE queue is FIFO and still busy
            # with the x slabs -- this copy must not wait behind them.
            nc.gpsimd.dma_start(out=qT[64:128, :], in_=qT[0:64, :])

        # ---- K projection stacks + transposes + attention slots ----
        # stack A: top = permuted cols 0-511 (chunks 0-3),
        #          bottom = cols 1024-1535 (chunks 8-11)   [slabs 0, 2]
        # stack B: top = 512-1023 (4-7), bottom = 1536-2047 (12-15) [1, 3]
        kstk = []
        vones = []
        o_done = []

        # slab for (stack, half): A=(s0 top, s2 bottom), B=(s1 top, s3 bottom)
        SLAB = {(0, 0): 0, (0, 1): 2, (1, 0): 1, (1, 1): 3}
        for si in range(2):
            kt = kqpool.tile([128, 512], bf16, name=f"kstk{si}")
            kstk.append(kt)
            vo = kqpool.tile([128, 8, D + 1], bf16, name=f"vones{si}")
            nc.vector.memset(vo[:, :, D : D + 1], 1.0)
            vones.append(vo)

        def kproj_half(si, half, q=None, cast_dve=False):
            """solo M=64 projection of one 512-col half into kstk[si].
            q selects a 256-col quarter (for the late B-bottom path)."""
            slab = xs[SLAB[(si, half)]]
            cs = slice(0, 512) if q is None else slice(q * 256, (q + 1) * 256)
            k_ps = psP.tile([128, 512], fp32, tag="proj",
                            name=f"kps{si}_{half}_{q}")
            hs = slice(64 * half, 64 * half + 64)
            for c in range(CCH):
                nc.tensor.matmul(
                    k_ps[hs, cs], lhsT=w_sb[:, c, 0:D], rhs=slab[:, c, cs],
                    start=(c == 0), stop=(c == CCH - 1),
                )
            if cast_dve:
                nc.vector.tensor_copy(kstk[si][hs, cs], k_ps[hs, cs])
            else:
                nc.scalar.copy(kstk[si][hs, cs], k_ps[hs, cs])

        def transp_half(si, half, only_p0=None):
            """V natural (+ones) for the 4 chunks of one half of stack si."""
            vo = vones[si]
            for p0 in ((0, 1) if only_p0 is None else (only_p0,)):
                pt2 = psP.tile(
                    [128, 128], bf16, tag="proj", name=f"tp{si}_{half}_{p0}"
                )
                for dk in range(2):
                    within = p0 * 2 + dk
                    nc.tensor.transpose(
                        pt2[:, dk * 64 : (dk + 1) * 64],
                        in_=kstk[si][64 * half : 64 * half + 64,
                                     within * CHUNK : (within + 1) * CHUNK],
                        identity=ident[64 * half : 64 * half + 64,
                                       64 * half : 64 * half + 64],
                    )
                w0 = half * 4 + p0 * 2
                nc.vector.tensor_copy(vo[:, w0 : w0 + 2, 0:D], pt2)

        def lhsT_of(c):
            si, half, within = _chunk_stack(c)
            return kstk[si][64 * half : 64 * half + 64,
                            within * CHUNK : (within + 1) * CHUNK]

        def vones_of(c):
            si, half, within = _chunk_stack(c)
            return vones[si][:, half * 4 + within, :]

        o_tiles = {}
        pt_tiles = {}

        def sexp_group(j, g):
            """S^T matmuls + exp (+ masks) for group g of slot j."""
            gch, kind = _slot_groups_def(j)[g]
            s_ps = psS.tile([128, GROUP * QTILE], fp32, tag="s",
                            name=f"sps{j}_{g}")
            order = (0, 2, 1, 3) if kind == "mixed" else (0, 1, 2, 3)
            for sl in order:
                cc = gch[sl]
                half = _chunk_stack(cc)[1]
                nc.tensor.matmul(
                    s_ps[:, sl * QTILE : (sl + 1) * QTILE],
                    lhsT=lhsT_of(cc),
                    rhs=qT[64 * half : 64 * half + 64,
                           j * QTILE : (j + 1) * QTILE],
                    start=True, stop=True,
                )
            pt = ptpool.tile([128, GROUP * QTILE], bf16, tag="pt", name=f"pt{j}_{g}")
            nc.scalar.activation(
                pt, s_ps, mybir.ActivationFunctionType.Exp, scale=SCALE
            )
            if kind == "mixed":
                nc.vector.tensor_mul(
                    pt[:, 0 : 2 * QTILE], pt[:, 0 : 2 * QTILE],
                    mask_sb[:, 2 * j : 2 * j + 2, :].rearrange("p g q -> p (g q)"),
                )
                nc.vector.tensor_scalar_mul(
                    pt[:, 2 * QTILE :], pt[:, 2 * QTILE :], scal_sb[:, j : j + 1]
                )
            elif kind == "own_diag":
                nc.vector.tensor_mul(
                    pt[:, 2 * QTILE :], pt[:, 2 * QTILE :],
                    mask_sb[:, 2 * j : 2 * j + 2, :].rearrange("p g q -> p (g q)"),
                )
            elif kind == "oth_tail":
                nc.vector.tensor_scalar_mul(
                    pt[:, 2 * QTILE :], pt[:, 2 * QTILE :], scal_sb[:, j : j + 1]
                )
            pt_tiles[(j, g)] = pt

        def pv_groups(j, glist):
            """PV accumulation for the given groups of slot j; finalizes
            (copy + DMA out) when the last group is included."""
            gdefs = _slot_groups_def(j)
            ngroups = len(gdefs)
            nch = ngroups * GROUP
            if j in o_tiles:
                o_ps = o_tiles[j]
            else:
                o_ps = psO.tile([D + 1, QTILE], fp32, tag="o", name=f"ops{j}")
                o_tiles[j] = o_ps
            for g in glist:
                gch, _ = gdefs[g]
                pt = pt_tiles.pop((j, g))
                for sl, cc in enumerate(gch):
                    k_abs = g * GROUP + sl
                    nc.tensor.matmul(
                        o_ps, lhsT=vones_of(cc),
                        rhs=pt[:, sl * QTILE : (sl + 1) * QTILE],
                        start=(k_abs == 0), stop=(k_abs == nch - 1),
                    )
            if glist[-1] == ngroups - 1:
                o_sb = opool.tile([D + 1, QTILE], fp32, name=f"osb{j}")
                nc.vector.tensor_copy(o_sb, o_ps)
                nc.gpsimd.dma_start(
                    out=out[:, j * QTILE : (j + 1) * QTILE], in_=o_sb
                )

        # emission order follows slab arrival: s0, s1, s2, s3
        kproj_half(0, 0)   # A-top    <- s0
        filler(12, "q")
        qproj()            # needs s0+s1
        kproj_half(1, 0)   # B-top    <- s1 (fills the s2 wait)
        # own-only S^T groups: need only A-top/B-top + qT -> exp starts early
        sexp_group(1, 0)   # {0,1,2,3}
        sexp_group(2, 0)
        sexp_group(3, 0)
        sexp_group(3, 2)   # {4,5,6,7} (B-top)
        transp_half(0, 0)
        transp_half(1, 0)
        filler(12, "ab")
        kproj_half(0, 1, cast_dve=True)   # A-bottom <- s2
        sexp_group(0, 0)   # {0,1,8,9}
        sexp_group(1, 1)   # {8..11}
        sexp_group(2, 1)
        sexp_group(3, 1)
        transp_half(0, 1)
        kproj_half(1, 1, q=0, cast_dve=True)   # B-bottom chunks 12,13
        sexp_group(2, 2)        # {4,5,12,13}
        transp_half(1, 1, only_p0=0)
        kproj_half(1, 1, q=1, cast_dve=True)   # chunks 14,15
        sexp_group(3, 3)        # {12..15}
        transp_half(1, 1, only_p0=1)
        pv_groups(0, [0])
        pv_groups(1, [0, 1])
        pv_groups(2, [0, 1, 2])
        pv_groups(3, [0, 1, 2, 3])

    nc.compile()
    return nc


_NC_CACHE = None


def _get_nc():
    global _NC_CACHE
    if _NC_CACHE is None:
        _NC_CACHE = _build_graph()
    return _NC_CACHE


def _perm_tiles(r):
    """permuted 256-col tile order: own tiles (2j+r) first, then others."""
    own = [2 * j + r for j in range(N_SLOTS)]
    oth = [2 * j + (1 - r) for j in range(N_SLOTS)]
    return own + oth


def _host_prep(x, W_Q, W_K):
    in_maps = []
    wkq = np.concatenate([W_K.T, W_Q.T], axis=1).astype(BF16)  # [1024, 128]
    pchunk = np.arange(CHUNK)
    f = np.arange(QTILE)
    for i in range(N_CORES):
        b, r = i % B, i // B
        perm = _perm_tiles(r)
        xt = x[b].T.astype(BF16)  # [1024, 2048]
        cols = np.concatenate(
            [np.arange(QTILE * p, QTILE * p + QTILE) for p in perm]
        )
        xkt = np.ascontiguousarray(xt[:, cols])
        # mask[p, 2j+h, f]: causal mask of own diag chunks (2j, 2j+1);
        # scal[p, j]: 0/1 multiplier for the other-side tail chunks
        m = np.zeros((CHUNK, 2 * N_SLOTS, QTILE), dtype=np.float32)
        sc = np.zeros((CHUNK, N_SLOTS), dtype=np.float32)
        for j in range(N_SLOTS):
            t_abs = QTILE * (2 * j + r) + f[None, :]
            for h in range(2):
                c = 2 * j + h               # own chunk -> tile 2j+r
                s_abs = QTILE * (2 * j + r) + CHUNK * h + pchunk[:, None]
                m[:, 2 * j + h, :] = (s_abs <= t_abs)
            # oth chunks 8+2j, 8+2j+1 -> original tile 2j+(1-r):
            # r=1 -> tile 2j < own tile 2j+1: fully valid (1.0)
            # r=0 -> tile 2j+1 > own tile 2j: fully masked (0.0)
            sc[:, j] = float(r)
        in_maps.append(
            {
                "xkt": xkt,
                "wkq": wkq,
                "mask": np.ascontiguousarray(m.astype(BF16)),
                "scal": np.ascontiguousarray(sc.astype(np.float32)),
            }
        )
    return in_maps


def _ensure_ntff_hook():
    """Install the antenv.axon_hooks shim so trace=True works under axon."""
    import types

    try:
        from antenv.axon_hooks import get_axon_ntff_profile_hook  # noqa: F401

        return
    except ImportError:
        pass
    import antenv

    mod = types.ModuleType("antenv.axon_hooks")
    mod._hook = None

    def set_axon_ntff_profile_hook(h):
        mod._hook = h

    def get_axon_ntff_profile_hook():
        return mod._hook

    mod.set_axon_ntff_profile_hook = set_axon_ntff_profile_hook
    mod.get_axon_ntff_profile_hook = get_axon_ntff_profile_hook
    sys.modules["antenv.axon_hooks"] = mod
    antenv.axon_hooks = mod
    try:
        from trn_agent_boot.trn_boot import _ntff_profile_via_ctypes

        hook = _ntff_profile_via_ctypes("/opt/axon/libaxon_pjrt.so")
        if hook is not None:
            set_axon_ntff_profile_hook(hook)
    except Exception as e:  # degrade to no tracing
        print(f"ntff hook install failed: {e}")


def kernel(x, W_Q, W_K, W_V=None, **_unused):
    global LAST_RESULTS
    if TRACE:
        _ensure_ntff_hook()
    x = np.asarray(x, dtype=np.float32)
    W_Q = np.asarray(W_Q, dtype=np.float32)
    W_K = np.asarray(W_K, dtype=np.float32)

    from concourse.bass_utils import run_bass_kernel_spmd

    nc = _get_nc()
    in_maps = _host_prep(x, W_Q, W_K)
    res = run_bass_kernel_spmd(
        nc,
        in_maps,
        core_ids=list(range(N_CORES)),
        trace=TRACE,
        trace_cores=TRACE_CORES,
    )
    LAST_RESULTS = res

    y = np.empty((B, T, D), dtype=np.float32)
    for i in range(N_CORES):
        b, r = i % B, i // B
        ot = res.results[i]["out"]  # [65, 1024]
        o = ot[0:D, :] / ot[D : D + 1, :]
        for j in range(N_SLOTS):
            t0 = QTILE * (2 * j + r)
            y[b, t0 : t0 + QTILE, :] = o[:, j * QTILE : (j + 1) * QTILE].T
    return y

